# revision 1
# baseline (speedup 1.0000x reference)
"""Trainium2 Bass kernel for nn_DecoderBlock (B=2, T=2048, D=1024, H=16, MLP=4096).

Sharding (2 groups of 4 cores, one group per batch element):
- Self-attention: HEAD-parallel. Every core receives the full x for its batch,
  computes ln1 over all T locally (no collective), then Q/K/V for its own 4
  heads over all T. The causal structure is identical on every core, so key
  chunks above the diagonal are skipped outright (~40% less attention work),
  with an SPMD-uniform program. Heads are exchanged back to row-parallel with
  a single 1 MB AllToAll per core.
- Cross-attention + MLP: sequence/row parallel (512 rows per core). Cross K/V
  depend only on enc_out, so they are computed first and their AllGathers
  launch immediately; the GpSimd queue carries nothing but collective
  triggers so the triggers are never delayed.

On-device layout: activations transposed [feature, row]; LayerNorm affine
folded into the following weight matrices host-side. Matmuls bf16, fp32
accumulation. Softmax without max-subtraction; denominator via a ones column
appended to V.
"""

import sys

if "/opt/trn_rl_repo" not in sys.path:
    sys.path.insert(0, "/opt/trn_rl_repo")

import numpy as np
import ml_dtypes

import concourse.bass as bass
import concourse.mybir as mybir
import concourse.tile as tile
from concourse import bacc
from concourse.bass_utils import run_bass_kernel_spmd

F32 = mybir.dt.float32
BF16 = mybir.dt.bfloat16

B, T, D, H, HD = 2, 2048, 1024, 16, 64
MLP = 4 * D
EPS = 1e-5
N_CORES = 8
GROUP = 4            # cores per batch element
R = T // GROUP       # rows per core = 512
DC = D // 128        # feature chunks = 8
MC = MLP // 128      # mlp chunks = 32
KC = T // 128        # key chunks = 16
NPAIR = H // 2       # head pairs (cross attn) = 8
HC = 2               # local head-pair chunks (self attn) = 2 (4 heads/core)
TC = T // R          # 512-wide column chunks of full T = 4
DH = D // 2
HH = H // 2
SCALE = HD ** -0.5


def build_program(trace_scopes=False):
    nc = bacc.Bacc("TRN2", target_bir_lowering=False, debug=False,
                   num_devices=N_CORES)

    # ---- DRAM I/O ----
    x_t = nc.dram_tensor("x_t", [D, R], F32, kind="ExternalInput")
    x_full = nc.dram_tensor("x_full", [D, T], F32, kind="ExternalInput")
    enc_tb = nc.dram_tensor("enc_tb", [D, R], BF16, kind="ExternalInput")
    mask_b = nc.dram_tensor("mask_b", [KC, 128, R], BF16, kind="ExternalInput")
    wqh = nc.dram_tensor("wqh", [DC, HC, 128, 128], BF16, kind="ExternalInput")
    wkh = nc.dram_tensor("wkh", [DC, HC, 128, 128], BF16, kind="ExternalInput")
    wvh = nc.dram_tensor("wvh", [DC, 128, HC * 128], BF16, kind="ExternalInput")
    wproj = nc.dram_tensor("wproj", [DC, DC, 128, 128], BF16, kind="ExternalInput")
    wq2 = nc.dram_tensor("wq2", [DC, DC, 128, 128], BF16, kind="ExternalInput")
    wkvk = nc.dram_tensor("wkvk", [DC, DC, 128, 128], BF16, kind="ExternalInput")
    wkvv = nc.dram_tensor("wkvv", [DC, 128, D], BF16, kind="ExternalInput")
    wco = nc.dram_tensor("wco", [DC, DC, 128, 128], BF16, kind="ExternalInput")
    wm1 = nc.dram_tensor("wm1", [DC, MC, 128, 128], BF16, kind="ExternalInput")
    wm2 = nc.dram_tensor("wm2", [MC, DC, 128, 128], BF16, kind="ExternalInput")
    bqh = nc.dram_tensor("bqh", [HC, 128], F32, kind="ExternalInput")
    bkh = nc.dram_tensor("bkh", [HC, 128], F32, kind="ExternalInput")
    bvh = nc.dram_tensor("bvh", [1, HC * 128], BF16, kind="ExternalInput")
    bproj = nc.dram_tensor("bproj", [DC, 128], F32, kind="ExternalInput")
    bq2 = nc.dram_tensor("bq2", [DC, 128], F32, kind="ExternalInput")
    bkvk = nc.dram_tensor("bkvk", [DC, 128], F32, kind="ExternalInput")
    bkvv = nc.dram_tensor("bkvv", [1, D], BF16, kind="ExternalInput")
    bco = nc.dram_tensor("bco", [DC, 128], F32, kind="ExternalInput")
    bm1 = nc.dram_tensor("bm1", [MC, 128], F32, kind="ExternalInput")
    bm2 = nc.dram_tensor("bm2", [DC, 128], F32, kind="ExternalInput")
    gmask = nc.dram_tensor("gmask", [128, 2 * GROUP], F32, kind="ExternalInput")
    out_t = nc.dram_tensor("out_t", [D, R], F32, kind="ExternalOutput")

    rg = [[0, 1, 2, 3], [4, 5, 6, 7]]

    with tile.TileContext(nc) as tc:
        with (
            tc.tile_pool(name="persist", bufs=1) as pp,
            tc.tile_pool(name="dram", bufs=1, space="DRAM") as dram,
        ):
            # cross-attn K^T+V packed per half: rows 0:DH = K^T, DH:2DH = V
            kvC_in = [dram.tile([2 * DH, R], BF16, name=f"kvCi{i}") for i in range(2)]
            kvC_out = [dram.tile([GROUP * 2 * DH, R], BF16, name=f"kvCo{i}")
                       for i in range(2)]
            # 4-rank AllToAll is unsupported (mesh needs >4 ranks), so the
            # head→row exchange runs as an 8-core A2A: chunks aimed at the
            # other batch-group are zeroed via a host-provided per-core mask
            # and the receiver adds the two candidate chunks (one is zero).
            a2a_in = [dram.tile([2 * GROUP * 128, R], BF16, name=f"a2ai{i}")
                      for i in range(HC)]
            a2a_out = [dram.tile([2 * GROUP * 128, R], BF16, name=f"a2ao{i}")
                       for i in range(HC)]
            rg_all = [list(range(N_CORES))]

            def allgather(src_d, dst_d):
                nc.gpsimd.collective_compute(
                    "AllGather", mybir.AluOpType.bypass,
                    ins=[src_d.opt()], outs=[dst_d.opt()], replica_groups=rg)

            def alltoall(src_d, dst_d):
                nc.gpsimd.collective_compute(
                    "AllToAll", mybir.AluOpType.bypass,
                    ins=[src_d.opt()], outs=[dst_d.opt()], replica_groups=rg_all)

            # persistent SBUF (whole program)
            x_sb = pp.tile([128, DC, R], F32)        # local residual x^T
            qt2_sb = pp.tile([128, DC, R], BF16)     # Q^T (cross)
            at_self = pp.tile([128, DC, R], BF16)    # attn out^T (self, post-A2A)
            at_cross = pp.tile([128, DC, R], BF16)   # attn out^T (cross)
            ones_full = pp.tile([128, 128], F32)
            ones_rbf = pp.tile([1, 128], BF16)
            ones_bf_col = pp.tile([128, 1], BF16)
            bias_sb = pp.tile([128, 2 * HC + 5 * DC + MC], F32)
            bv_sb = pp.tile([1, HC * 128], BF16)
            bkvv_sb = pp.tile([1, D], BF16)
            eps_sb = pp.tile([1, 1], F32)
            gm_sb = pp.tile([128, 2 * GROUP], F32)

            nc.vector.memset(ones_full[:], 1.0)
            nc.vector.memset(ones_rbf[:], 1.0)
            nc.vector.memset(ones_bf_col[:], 1.0)
            nc.vector.memset(eps_sb[:], EPS)

            nc.sync.dma_start(x_sb[:], x_t.ap().rearrange("(c p) f -> p c f", p=128))
            nc.sync.dma_start(gm_sb[:], gmask.ap())
            nc.sync.dma_start(bv_sb[:], bvh.ap())
            nc.sync.dma_start(bkvv_sb[:], bkvv.ap())
            # biases: bqh, bkh [HC,128] then 5 arrays of [DC,128] then bm1 [MC,128]
            nc.sync.dma_start(bias_sb[:, 0:HC], bqh.ap().rearrange("c p -> p c"))
            nc.sync.dma_start(bias_sb[:, HC:2 * HC], bkh.ap().rearrange("c p -> p c"))
            off = 2 * HC
            for i, b in enumerate([bproj, bq2, bkvk, bco, bm2]):
                nc.sync.dma_start(
                    bias_sb[:, off + i * DC:off + (i + 1) * DC],
                    b.ap().rearrange("c p -> p c"))
            nc.sync.dma_start(
                bias_sb[:, off + 5 * DC:off + 5 * DC + MC],
                bm1.ap().rearrange("c p -> p c"))
            B_Q, B_K = 0, HC
            B_PROJ, B_Q2, B_KVK, B_CO, B_M2 = (off, off + DC, off + 2 * DC,
                                               off + 3 * DC, off + 4 * DC)
            B_M1 = off + 5 * DC

            def bias_ap(base, oc):
                return bias_sb[:, base + oc:base + oc + 1]

            # ---------- helpers ----------
            def layernorm_into(src_sb, dst_ap, pool, psum_pool, name):
                """src_sb [128, DC, W] f32 -> dst_ap [128, DC, W] bf16."""
                W = src_sb.shape[-1]
                ps1 = psum_pool.tile([1, W], F32, tag="stats", bufs=2,
                                     name=f"p1_{name}")
                ps2 = psum_pool.tile([1, W], F32, tag="stats", bufs=2,
                                     name=f"p2_{name}")
                for c in range(DC):
                    sq = pool.tile([128, W], BF16, tag="lnsq", name=f"sq_{name}{c}")
                    xb = pool.tile([128, W], BF16, tag="lnxb", name=f"xb_{name}{c}")
                    nc.scalar.square(sq[:], src_sb[:, c, :])
                    nc.scalar.copy(xb[:], src_sb[:, c, :])
                    nc.tensor.matmul(ps1[:], lhsT=ones_bf_col[:], rhs=xb[:],
                                     start=(c == 0), stop=(c == DC - 1))
                    nc.tensor.matmul(ps2[:], lhsT=ones_bf_col[:], rhs=sq[:],
                                     start=(c == 0), stop=(c == DC - 1))
                nmean = pool.tile([1, W], F32, tag="lnrow", bufs=8, name=f"nm_{name}")
                ex2 = pool.tile([1, W], F32, tag="lnrow", bufs=8, name=f"e2_{name}")
                m2 = pool.tile([1, W], F32, tag="lnrow", bufs=8, name=f"m2_{name}")
                var = pool.tile([1, W], F32, tag="lnrow", bufs=8, name=f"va_{name}")
                std = pool.tile([1, W], F32, tag="lnrow", bufs=8, name=f"sd_{name}")
                rstd = pool.tile([1, W], F32, tag="lnrow", bufs=8, name=f"rs_{name}")
                nmrs = pool.tile([1, W], F32, tag="lnrow", bufs=8, name=f"nr_{name}")
                nc.scalar.activation(nmean[:], ps1[:],
                                     mybir.ActivationFunctionType.Identity,
                                     scale=-1.0 / D)
                nc.scalar.activation(ex2[:], ps2[:],
                                     mybir.ActivationFunctionType.Identity,
                                     scale=1.0 / D)
                nc.vector.tensor_tensor(m2[:], nmean[:], nmean[:],
                                        mybir.AluOpType.mult)
                nc.vector.tensor_tensor(var[:], ex2[:], m2[:],
                                        mybir.AluOpType.subtract)
                # Sqrt's table set also holds Square/Identity (fillers), so
                # LN causes no ACT table churn; the reciprocal runs on DVE.
                nc.scalar.activation(std[:], var[:],
                                     mybir.ActivationFunctionType.Sqrt,
                                     bias=eps_sb[:])
                nc.vector.reciprocal_approx_fast(rstd[:], std[:])
                nc.vector.tensor_tensor(nmrs[:], nmean[:], rstd[:],
                                        mybir.AluOpType.mult)
                psb = psum_pool.tile([128, 2 * W], F32, tag="lnb", bufs=1,
                                     name=f"pb_{name}")
                nc.tensor.matmul(psb[:, 0:W], lhsT=ones_full[0:1, :],
                                 rhs=rstd[:], start=True, stop=True)
                nc.tensor.matmul(psb[:, W:2 * W], lhsT=ones_full[0:1, :],
                                 rhs=nmrs[:], start=True, stop=True)
                tmp = pool.tile([128, W], F32, tag="lntmp", name=f"tp_{name}")
                for c in range(DC):
                    nc.vector.tensor_tensor(tmp[:], src_sb[:, c, :], psb[:, 0:W],
                                            mybir.AluOpType.mult)
                    nc.vector.tensor_tensor(dst_ap[:, c, :], tmp[:], psb[:, W:2 * W],
                                            mybir.AluOpType.add)

            def layernorm(src_sb, pool, psum_pool, name):
                out = pool.tile([128, DC, R], BF16, tag="lnT", name=f"ln_{name}")
                layernorm_into(src_sb, out, pool, psum_pool, name)
                return out

            def matmul_t(rhs_sb, w_dram, n_k, n_o, pool, psum_pool, name,
                         consume, w_tag="wtile", ocs=None, kcs=None):
                """out^T[oc] = sum_kc W[kc,oc].T @ rhs[kc]."""
                kcs = list(range(n_k)) if kcs is None else kcs
                for oc in (range(n_o) if ocs is None else ocs):
                    wt = pool.tile([128, n_k, 128], BF16, tag=w_tag,
                                   name=f"w_{name}_{oc}")
                    nc.gpsimd.dma_start(
                        wt[:], w_dram.ap()[:, oc].rearrange("k p m -> p k m"))
                    ps = psum_pool.tile([128, R], F32, tag="mm", name=f"ps_{name}_{oc}")
                    for i, kc in enumerate(kcs):
                        nc.tensor.matmul(ps[:], lhsT=wt[:, kc, :],
                                         rhs=rhs_sb[:, kc, :],
                                         start=(i == 0), stop=(i == n_k - 1))
                    consume(oc, ps)

            def softmax_pv(pss, es_shape_w, vf_kc_aps, psA, psB, kc, nkc, pool,
                           psum_pool, name, mask_ap=None):
                """exp + optional mask + PV accumulation for one key chunk."""
                w = es_shape_w
                es = pool.tile([128, 2, w], BF16, tag="expS", name=f"e_{name}")
                nc.scalar.activation(
                    es[:].rearrange("p a f -> p (a f)"), pss[:],
                    mybir.ActivationFunctionType.Exp, scale=SCALE)
                if mask_ap is not None:
                    nc.vector.tensor_tensor(
                        es[:], es[:], mask_ap.to_broadcast((128, 2, w)),
                        mybir.AluOpType.mult)
                nc.tensor.matmul(psA[:], lhsT=vf_kc_aps[0], rhs=es[:, 0, :],
                                 start=(kc == 0), stop=(kc == nkc - 1))
                nc.tensor.matmul(psB[:], lhsT=vf_kc_aps[1], rhs=es[:, 1, :],
                                 start=(kc == 0), stop=(kc == nkc - 1))

            def normalize(psA, psB, w, dst0, dst1, pool, psum_pool, name):
                """Divide accumulated PV by the denominator row (partition HD)."""
                for hh, pso, dst in ((0, psA, dst0), (1, psB, dst1)):
                    rec = pool.tile([HD + 1, w], F32, tag="rec",
                                    name=f"r_{name}_{hh}")
                    nc.vector.tensor_copy(rec[HD:HD + 1, :], pso[HD:HD + 1, :])
                    pbig = psum_pool.tile([128, 2 * w], F32, tag="psS",
                                          bufs=2, name=f"b_{name}_{hh}")
                    pbc = pbig[0:HD, 0:w]
                    nc.tensor.matmul(pbc, lhsT=ones_full[HD:HD + 1, 0:HD],
                                     rhs=rec[HD:HD + 1, :], start=True, stop=True)
                    bcs = pool.tile([HD, w], F32, tag="bcs", name=f"c_{name}_{hh}")
                    nc.vector.reciprocal_approx_fast(bcs[:], pbc)
                    if hh == 0:
                        nc.vector.tensor_tensor(dst, pso[0:HD, :], bcs[:],
                                                mybir.AluOpType.mult)
                    else:
                        tmb = pool.tile([HD, w], BF16, tag="tmb",
                                        name=f"t_{name}")
                        nc.vector.tensor_tensor(tmb[:], pso[0:HD, :], bcs[:],
                                                mybir.AluOpType.mult)
                        nc.sync.dma_start(dst, tmb[:])

            def attention_cross(qt, kv_halves, dst_sb, pool, psum_pool, name):
                """Row-parallel attention against gathered K^T/V (no mask)."""
                for hp in range(NPAIR):
                    half, hpl = hp // (NPAIR // 2), hp % (NPAIR // 2)
                    kv_out_d = kv_halves[half]
                    ktp = pool.tile([128, KC, 128], BF16, tag="ktp",
                                    name=f"kt_{name}_{hp}")
                    bpc = R // 128
                    for r in range(GROUP):
                        base = r * 2 * DH
                        nc.sync.dma_start(
                            ktp[:, r * bpc:(r + 1) * bpc, :],
                            kv_out_d[base + hpl * 128:base + (hpl + 1) * 128, :]
                            .rearrange("p (c m) -> p c m", m=128))
                    vts = []
                    for hh in range(2):
                        hl = 2 * hpl + hh
                        vt = pool.tile([128, KC, HD + 1], BF16, tag="vt", bufs=3,
                                       name=f"v_{name}_{2 * hp + hh}")
                        for r in range(GROUP):
                            base = r * 2 * DH + DH
                            nc.sync.dma_start(
                                vt[:, r * bpc:(r + 1) * bpc, 0:HD],
                                kv_out_d[base:base + R, hl * HD:(hl + 1) * HD]
                                .rearrange("(c p) d -> p c d", p=128))
                        nc.vector.memset(vt[:, :, HD:HD + 1], 1.0)
                        vts.append(vt)
                    psA = psum_pool.tile([HD + 1, R], F32, tag="psO", bufs=4,
                                         name=f"oA_{name}_{hp}")
                    psB = psum_pool.tile([HD + 1, R], F32, tag="psO", bufs=4,
                                         name=f"oB_{name}_{hp}")
                    for kc in range(KC):
                        pss = psum_pool.tile([128, 2, R], F32, tag="psS", bufs=2,
                                             name=f"s_{name}_{hp}_{kc}")
                        nc.tensor.matmul(pss[:, 0, :], lhsT=ktp[0:64, kc, :],
                                         rhs=qt[0:64, hp, :],
                                         start=True, stop=True)
                        nc.tensor.matmul(pss[:, 1, :], lhsT=ktp[64:128, kc, :],
                                         rhs=qt[64:128, hp, :],
                                         start=True, stop=True)
                        softmax_pv(pss, R,
                                   [vts[0][:, kc, :], vts[1][:, kc, :]],
                                   psA, psB, kc, KC, pool, psum_pool,
                                   f"{name}_{hp}_{kc}")
                    normalize(psA, psB, R,
                              dst_sb[0:HD, hp, :], dst_sb[HD:128, hp, :],
                              pool, psum_pool, f"{name}_{hp}")

            # ============ phases 0-2: ln1(full), cross KV + AGs, self QKV,
            # ============ head-parallel causal self-attention, A2A ============
            with tc.tile_pool(name="pA", bufs=1) as pa:
                enc_sb = pa.tile([128, DC, R], BF16)
                ln1f = pa.tile([128, DC, T], BF16)       # ln1 over all T
                ktf = pa.tile([128, HC, KC, 128], BF16)  # K^T my heads, all T
                qtf = pa.tile([128, HC, T], BF16)        # Q^T my heads, all T
                vf = pa.tile([128, KC, 2 * HC, HD + 1], BF16)  # V my heads
                at_h = pa.tile([128, HC, T], BF16)       # self out^T my heads
                wv_sb = pa.tile([128, DC, HC * 128], BF16)

                nc.sync.dma_start(enc_sb[:],
                                  enc_tb.ap().rearrange("(c p) f -> p c f", p=128))
                nc.sync.dma_start(wv_sb[:],
                                  wvh.ap().rearrange("k p m -> p k m"))
                nc.vector.memset(vf[:, :, :, HD:HD + 1], 1.0)

                with (
                    tc.tile_pool(name="p1", bufs=2) as pool,
                    tc.tile_pool(name="p1ps", bufs=2, space="PSUM") as psum_pool,
                ):
                    # prefetch the x chunks for ln1 first on the sync queue
                    LW = 256
                    xcs = []
                    for tcc in range(T // LW):
                        xc = pool.tile([128, DC, LW], F32, tag="xc", bufs=3,
                                       name=f"xc{tcc}")
                        nc.sync.dma_start(
                            xc[:],
                            x_full.ap().rearrange("(c p) f -> p c f", p=128)
                            [:, :, tcc * LW:(tcc + 1) * LW])
                        xcs.append(xc)

                    # cross K/V (local rows) + AllGathers come first in the PE
                    # queue (independent of ln1); bias-adds run on the idle
                    # Scalar engine so the Vector engine is free for ln1.
                    def eat_kc(oc, ps):
                        kl = pool.tile([128, R], BF16, tag="kvcopy",
                                       name=f"kc_{oc}")
                        nc.scalar.activation(kl[:], ps[:],
                                             mybir.ActivationFunctionType.Identity,
                                             bias=bias_ap(B_KVK, oc))
                        half, ocl = oc // (DC // 2), oc % (DC // 2)
                        nc.sync.dma_start(
                            kvC_in[half][ocl * 128:(ocl + 1) * 128, :], kl[:])

                    for half in range(2):
                        matmul_t(enc_sb, wkvk, DC, DC, pool, psum_pool,
                                 f"kc{half}", eat_kc,
                                 ocs=range(half * (DC // 2),
                                           (half + 1) * (DC // 2)))
                        sl = slice(half * DH, half * DH + DH)
                        wvv_sb = pool.tile([128, DC, DH], BF16, tag="wvv",
                                           bufs=1, name=f"wvv{half}")
                        nc.gpsimd.dma_start(
                            wvv_sb[:],
                            wkvv.ap()[:, :, sl].rearrange("k p m -> p k m"))
                        for rc in range(R // 128):
                            ps = psum_pool.tile([128, DH], F32, tag="pswide",
                                                bufs=2, name=f"vc{half}_{rc}")
                            for kc in range(DC):
                                nc.tensor.matmul(
                                    ps[:],
                                    lhsT=enc_sb[:, kc, rc * 128:(rc + 1) * 128],
                                    rhs=wvv_sb[:, kc, :],
                                    start=(kc == 0), stop=False)
                            nc.tensor.matmul(ps[:], lhsT=ones_rbf[:],
                                             rhs=bkvv_sb[:, sl],
                                             start=False, stop=True)
                            vl = pool.tile([128, DH], BF16, tag="vcopy",
                                           name=f"vc{half}_{rc}")
                            nc.scalar.copy(vl[:], ps[:])
                            nc.sync.dma_start(
                                kvC_in[half][DH + rc * 128:DH + (rc + 1) * 128, :],
                                vl[:])
                        allgather(kvC_in[half], kvC_out[half])

                    # self K/Q weights for both head-chunks, resident throughout
                    wtks, wtqs = [], []
                    for hc in range(HC):
                        wtk = pool.tile([128, DC, 128], BF16, tag="wtileS",
                                        bufs=4, name=f"wk_{hc}")
                        nc.gpsimd.dma_start(
                            wtk[:], wkh.ap()[:, hc].rearrange("k p m -> p k m"))
                        wtq = pool.tile([128, DC, 128], BF16, tag="wtileS",
                                        bufs=4, name=f"wq_{hc}")
                        nc.gpsimd.dma_start(
                            wtq[:], wqh.ap()[:, hc].rearrange("k p m -> p k m"))
                        wtks.append(wtk)
                        wtqs.append(wtq)

                    # ln1 and self K^T/Q^T/V interleaved per 512-column block,
                    # so the PE chews this block's QKV matmuls while the
                    # DVE/ACT LayerNorm chain works on the next block
                    for tcc in range(TC):
                        for sub in range(R // LW):
                            i = tcc * (R // LW) + sub
                            layernorm_into(xcs[i],
                                           ln1f[:, :, i * LW:(i + 1) * LW],
                                           pool, psum_pool, f"ln1_{i}")
                        for hc in range(HC):
                            psk = psum_pool.tile([128, R], F32, tag="mm",
                                                 name=f"psk_{hc}_{tcc}")
                            for kc in range(DC):
                                nc.tensor.matmul(
                                    psk[:], lhsT=wtks[hc][:, kc, :],
                                    rhs=ln1f[:, kc, tcc * R:(tcc + 1) * R],
                                    start=(kc == 0), stop=(kc == DC - 1))
                            nc.vector.tensor_scalar_add(
                                ktf[:, hc, tcc * (R // 128):(tcc + 1) * (R // 128), :]
                                .rearrange("p c m -> p (c m)"),
                                psk[:], bias_ap(B_K, hc))
                            psq = psum_pool.tile([128, R], F32, tag="mm",
                                                 name=f"psq_{hc}_{tcc}")
                            for kc in range(DC):
                                nc.tensor.matmul(
                                    psq[:], lhsT=wtqs[hc][:, kc, :],
                                    rhs=ln1f[:, kc, tcc * R:(tcc + 1) * R],
                                    start=(kc == 0), stop=(kc == DC - 1))
                            nc.scalar.activation(
                                qtf[:, hc, tcc * R:(tcc + 1) * R], psq[:],
                                mybir.ActivationFunctionType.Identity,
                                bias=bias_ap(B_Q, hc))
                        for rc in range(4 * tcc, 4 * (tcc + 1)):
                            ps = psum_pool.tile([128, HC * 128], F32,
                                                tag="pswide", bufs=2,
                                                name=f"psv_{rc}")
                            for kc in range(DC):
                                nc.tensor.matmul(
                                    ps[:],
                                    lhsT=ln1f[:, kc, rc * 128:(rc + 1) * 128],
                                    rhs=wv_sb[:, kc, :],
                                    start=(kc == 0), stop=False)
                            nc.tensor.matmul(ps[:], lhsT=ones_rbf[:],
                                             rhs=bv_sb[:],
                                             start=False, stop=True)
                            nc.vector.tensor_copy(
                                vf[:, rc, :, 0:HD],
                                ps[:].rearrange("p (h d) -> p h d", d=HD))

                # ---- head-parallel causal self-attention ----
                with (
                    tc.tile_pool(name="p2", bufs=2) as pool,
                    tc.tile_pool(name="p2ps", bufs=2, space="PSUM") as psum_pool,
                ):
                    mask_sb = pool.tile([128, KC, R], BF16, tag="mask", bufs=1)
                    nc.sync.dma_start(mask_sb[:],
                                      mask_b.ap().rearrange("c p f -> p c f"))
                    for hc in range(HC):
                        for qc in range(TC):
                            qsl = slice(qc * R, (qc + 1) * R)
                            nkc = 4 * (qc + 1)
                            psA = psum_pool.tile([HD + 1, R], F32, tag="psO",
                                                 bufs=4, name=f"oA_{hc}_{qc}")
                            psB = psum_pool.tile([HD + 1, R], F32, tag="psO",
                                                 bufs=4, name=f"oB_{hc}_{qc}")
                            for kc in range(nkc):
                                pss = psum_pool.tile([128, 2, R], F32, tag="psS",
                                                     bufs=2,
                                                     name=f"s_{hc}_{qc}_{kc}")
                                nc.tensor.matmul(pss[:, 0, :],
                                                 lhsT=ktf[0:64, hc, kc, :],
                                                 rhs=qtf[0:64, hc, qsl],
                                                 start=True, stop=True)
                                nc.tensor.matmul(pss[:, 1, :],
                                                 lhsT=ktf[64:128, hc, kc, :],
                                                 rhs=qtf[64:128, hc, qsl],
                                                 start=True, stop=True)
                                mask_ap = (mask_sb[:, kc, None, :]
                                           if kc >= 4 * qc else None)
                                softmax_pv(pss, R,
                                           [vf[:, kc, 2 * hc, :],
                                            vf[:, kc, 2 * hc + 1, :]],
                                           psA, psB, kc, nkc, pool, psum_pool,
                                           f"sa_{hc}_{qc}_{kc}", mask_ap)
                            normalize(psA, psB, R,
                                      at_h[0:HD, hc, qsl], at_h[HD:128, hc, qsl],
                                      pool, psum_pool, f"sa_{hc}_{qc}")
                            # stage this row-block for the AllToAll right away
                            # (group-masked; one chunk per destination rank)
                            for j in (qc, qc + GROUP):
                                row = j * 128
                                st = pool.tile([128, R], BF16, tag="a2st",
                                               bufs=4, name=f"a2s_{hc}_{j}")
                                nc.vector.tensor_scalar_mul(
                                    st[:], at_h[:, hc, qsl], gm_sb[:, j:j + 1])
                                nc.sync.dma_start(
                                    a2a_in[hc][row:row + 128, :], st[:])
                        # the first A2A and its readback overlap the second
                        # head-chunk's attention; proj can then start on the
                        # even at_self chunks while the second A2A is in flight
                        alltoall(a2a_in[hc], a2a_out[hc])
                        for i in range(GROUP):
                            rowa = i * 128
                            rowb = (i + GROUP) * 128
                            ta = pool.tile([128, R], BF16, tag="a2r", bufs=4,
                                           name=f"a2ra_{i}_{hc}")
                            tb = pool.tile([128, R], BF16, tag="a2r", bufs=4,
                                           name=f"a2rb_{i}_{hc}")
                            nc.sync.dma_start(ta[:], a2a_out[hc][rowa:rowa + 128, :])
                            nc.sync.dma_start(tb[:], a2a_out[hc][rowb:rowb + 128, :])
                            nc.vector.tensor_tensor(at_self[:, i * HC + hc, :],
                                                    ta[:], tb[:],
                                                    mybir.AluOpType.add)

            # ============ phase 3: proj + residual, ln2, q2 ============
            with tc.tile_pool(name="pB", bufs=1) as pb:
                x2_sb = pb.tile([128, DC, R], F32)
                x3_sb = pb.tile([128, DC, R], F32)

                with (
                    tc.tile_pool(name="p3", bufs=2) as pool,
                    tc.tile_pool(name="p3ps", bufs=2, space="PSUM") as psum_pool,
                ):
                    def eat_proj(oc, ps):
                        nc.vector.scalar_tensor_tensor(
                            x2_sb[:, oc, :], ps[:], bias_ap(B_PROJ, oc),
                            x_sb[:, oc, :],
                            mybir.AluOpType.add, mybir.AluOpType.add)

                    matmul_t(at_self, wproj, DC, DC, pool, psum_pool, "pr",
                             eat_proj, kcs=[0, 2, 4, 6, 1, 3, 5, 7])

                    ln2 = layernorm(x2_sb, pool, psum_pool, "ln2")

                    def eat_q2(oc, ps):
                        nc.scalar.activation(
                            qt2_sb[:, oc, :], ps[:],
                            mybir.ActivationFunctionType.Identity,
                            bias=bias_ap(B_Q2, oc))

                    matmul_t(ln2, wq2, DC, DC, pool, psum_pool, "q2", eat_q2)

                # ============ phase 4: cross attention ============
                with (
                    tc.tile_pool(name="p4", bufs=2) as pool4,
                    tc.tile_pool(name="p4ps", bufs=2, space="PSUM") as psum4,
                ):
                    attention_cross(qt2_sb, kvC_out, at_cross, pool4, psum4, "ca")

                # ============ phase 5: co + residual, ln3, MLP ============
                with (
                    tc.tile_pool(name="p5", bufs=2) as pool5,
                    tc.tile_pool(name="p5ps", bufs=2, space="PSUM") as psum5,
                ):
                    def eat_co(oc, ps):
                        nc.vector.scalar_tensor_tensor(
                            x3_sb[:, oc, :], ps[:], bias_ap(B_CO, oc),
                            x2_sb[:, oc, :],
                            mybir.AluOpType.add, mybir.AluOpType.add)

                    matmul_t(at_cross, wco, DC, DC, pool5, psum5, "co", eat_co)

                    ln3 = layernorm(x3_sb, pool5, psum5, "ln3")

                    h_sb = pool5.tile([128, MC, R], BF16, tag="hsb", bufs=1)

                    def eat_m1(oc, ps):
                        nc.scalar.activation(h_sb[:, oc, :], ps[:],
                                             mybir.ActivationFunctionType.Gelu,
                                             bias=bias_ap(B_M1, oc))

                    matmul_t(ln3, wm1, DC, MC, pool5, psum5, "m1", eat_m1)

                    # x_sb is dead after proj — reuse as the output buffer
                    def eat_m2(oc, ps):
                        nc.vector.scalar_tensor_tensor(
                            x_sb[:, oc, :], ps[:], bias_ap(B_M2, oc),
                            x3_sb[:, oc, :],
                            mybir.AluOpType.add, mybir.AluOpType.add)
                        nc.sync.dma_start(
                            out_t.ap().rearrange("(c p) f -> p c f", p=128)[:, oc, :],
                            x_sb[:, oc, :])

                    matmul_t(h_sb, wm2, MC, DC, pool5, psum5, "m2", eat_m2,
                             w_tag="wtile2")

    nc.finalize()
    return nc


def prep_inputs(inputs):
    """Host-side prep: fold LN affine into weights, cast/tile, shard."""
    f32 = np.float32
    bf16 = ml_dtypes.bfloat16

    def tile_w(w, nk, no):
        return np.ascontiguousarray(
            w.reshape(nk, 128, no, 128).transpose(0, 2, 1, 3)).astype(bf16)

    def chunk_b(b, n):
        return np.ascontiguousarray(b.reshape(n, 128)).astype(f32)

    x = np.asarray(inputs["x"], f32)
    enc = np.asarray(inputs["enc_out"], f32)
    cm = np.asarray(inputs["causal_mask"])

    ln1_g, ln1_b = np.asarray(inputs["ln1_g"], f32), np.asarray(inputs["ln1_b"], f32)
    ln2_g, ln2_b = np.asarray(inputs["ln2_g"], f32), np.asarray(inputs["ln2_b"], f32)
    ln3_g, ln3_b = np.asarray(inputs["ln3_g"], f32), np.asarray(inputs["ln3_b"], f32)
    qkv_w = np.asarray(inputs["qkv_w"], f32)
    qkv_b = np.asarray(inputs["qkv_b"], f32)
    q_w, q_b = np.asarray(inputs["q_w"], f32), np.asarray(inputs["q_b"], f32)
    kv_w, kv_b = np.asarray(inputs["kv_w"], f32), np.asarray(inputs["kv_b"], f32)
    mlp1_w, mlp1_b = np.asarray(inputs["mlp1_w"], f32), np.asarray(inputs["mlp1_b"], f32)

    qkv_w_eff = ln1_g[:, None] * qkv_w
    qkv_b_eff = qkv_b + ln1_b @ qkv_w
    q_w_eff = ln2_g[:, None] * q_w
    q_b_eff = q_b + ln2_b @ q_w
    m1_w_eff = ln3_g[:, None] * mlp1_w
    m1_b_eff = mlp1_b + ln3_b @ mlp1_w

    # causal diagonal band: band[i] = mask[keys of chunk i, queries of
    # chunk i//4] transposed to [128 keys, 512 queries]
    cmn = (cm != 0).astype(f32)
    band = np.stack([
        np.ascontiguousarray(cmn[R * (i // 4):R * (i // 4) + R,
                                 128 * i:128 * (i + 1)].T)
        for i in range(KC)
    ]).astype(bf16)

    shared = {
        "wproj": tile_w(np.asarray(inputs["proj_w"], f32), DC, DC),
        "wq2": tile_w(q_w_eff, DC, DC),
        "wkvk": tile_w(kv_w[:, 0:D], DC, DC),
        "wkvv": np.ascontiguousarray(
            kv_w[:, D:2 * D].reshape(DC, 128, D)).astype(bf16),
        "wco": tile_w(np.asarray(inputs["co_w"], f32), DC, DC),
        "wm1": tile_w(m1_w_eff, DC, MC),
        "wm2": tile_w(np.asarray(inputs["mlp2_w"], f32), MC, DC),
        "bproj": chunk_b(np.asarray(inputs["proj_b"], f32), DC),
        "bq2": chunk_b(q_b_eff, DC),
        "bkvk": chunk_b(kv_b[0:D], DC),
        "bkvv": kv_b[D:2 * D].reshape(1, D).astype(bf16),
        "bco": chunk_b(np.asarray(inputs["co_b"], f32), DC),
        "bm1": chunk_b(m1_b_eff, MC),
        "bm2": chunk_b(np.asarray(inputs["mlp2_b"], f32), DC),
        "mask_b": band,
    }

    in_maps = []
    for c in range(N_CORES):
        b = c // GROUP
        r = c % GROUP
        r0 = r * R
        hsl = slice(256 * r, 256 * r + 256)   # this core's 4 heads
        gm = np.zeros((128, 2 * GROUP), f32)
        gm[:, b * GROUP:(b + 1) * GROUP] = 1.0
        m = dict(shared)
        m["gmask"] = gm
        m["x_t"] = np.ascontiguousarray(x[b, r0:r0 + R].T)
        m["x_full"] = np.ascontiguousarray(x[b].T)
        m["enc_tb"] = np.ascontiguousarray(enc[b, r0:r0 + R].T).astype(bf16)
        m["wqh"] = tile_w(qkv_w_eff[:, 0:D][:, hsl], DC, HC)
        m["wkh"] = tile_w(qkv_w_eff[:, D:2 * D][:, hsl], DC, HC)
        m["wvh"] = np.ascontiguousarray(
            qkv_w_eff[:, 2 * D:3 * D][:, hsl].reshape(DC, 128, HC * 128)
        ).astype(bf16)
        m["bqh"] = chunk_b(qkv_b_eff[0:D][hsl], HC)
        m["bkh"] = chunk_b(qkv_b_eff[D:2 * D][hsl], HC)
        m["bvh"] = qkv_b_eff[2 * D:3 * D][hsl].reshape(1, HC * 128).astype(bf16)
        in_maps.append(m)
    return in_maps


_prog_cache = {}


def kernel(**inputs):
    if "nc" not in _prog_cache:
        _prog_cache["nc"] = build_program()
    nc = _prog_cache["nc"]
    in_maps = prep_inputs(inputs)
    res = run_bass_kernel_spmd(nc, in_maps, core_ids=list(range(N_CORES)))
    out = np.empty((B, T, D), np.float32)
    for c in range(N_CORES):
        b = c // GROUP
        r0 = (c % GROUP) * R
        out[b, r0:r0 + R] = res.results[c]["out_t"].T
    _prog_cache["last_results"] = res
    return out

